# revision 29
# baseline (speedup 1.0000x reference)
"""Trainium2 Bass kernel for CausalSequenceCML.

Math (reference, per step, grid g laid out (B, C, T)):
    mapped  = r * g * (1 - g)
    local   = causal depthwise conv1d(mapped, K, left pad 3)   # per channel
    physics = (1 - eps) * mapped + eps * local
    g'      = (1 - beta) * physics + beta * x0                 # x0 = initial grid

Because r, eps, beta, K are per-channel constants and the conv is linear,
the whole update is affine in a = g*(1-g) = 0.25 - (g-0.5)^2:

    g' = D - C3*sq[t] - C2*sq[t-1] - C1*sq[t-2] - C0*sq[t-3]
    sq = (g - 0.5)^2
    Cj = (1-beta)*eps*r*K[j]             (j = 0, 1, 2)
    C3 = (1-beta)*r*((1-eps) + eps*K[3])
    D  = beta*x0 + 0.25*(C0+C1+C2+C3)

Left-boundary: conv pads mapped (=> a) with zeros, i.e. sq = 0.25 there; the
sq buffer has 3 leading pad columns held at 0.25.

Sharding: C=512 split across 8 cores (64 ch each). Per core the (B=4, 64, T)
block is flattened to 256 rows = 2 SBUF tiles of (128, 4096), channels+batch
on partitions, time on the free dim.

Engine split, per step per tile (all fp32 — the chaotic map amplifies
rounding ~1000x over 16 steps, so 16-bit or float32r anywhere fails):
 - ScalarE: sq = Square(g - 0.5) into the padded sq buffer.
 - VectorE: columns [0, PE_SPLIT) via 4 fused scalar_tensor_tensor ops
   (per-partition coeff multiply + accumulate), runs at 1 elem/cycle.
 - TensorE: columns [PE_SPLIT, T) via 5 PSUM-accumulated fp32 matmuls per
   512-col block: W = diag(-Cj) for the 4 taps (time shifts come free via
   the moving operand's AP column offset) plus an identity-diagonal matmul
   that adds D — so PSUM holds g' directly (fp32 matmul is 2-pass LO/HI,
   measured exact to 1e-7).
 - ScalarE copies the finished PSUM blocks to the state buffer.
GPSIMD stays idle: its SBUF port is an exclusive lock shared with DVE's
2-port ops, so concurrent GPSIMD work measured 2.8x slower overall.
"""

import numpy as np

from concourse import bacc, mybir
import concourse.tile as tile
from concourse.bass_utils import run_bass_kernel_spmd

B, T, C = 4, 4096, 512
N_CORES = 8
CPC = C // N_CORES          # channels per core = 64
ROWS = B * CPC              # 256 rows per core
HALVES = ROWS // 128        # 2 SBUF tiles per core
CLAMP = 1e-4
F32 = mybir.dt.float32

# Column split: DVE handles [0, SPLIT), GPSIMD handles [SPLIT, T) of every
# accumulation op. SPLIT = T disables GPSIMD. Measured on HW: GPSIMD fully
# blocks behind DVE's 2-port-mode ops (shared SBUF port pair, exclusive
# lock), so any split is a large net loss — keep SPLIT = T.
SPLIT = T

# Interleave the two tiles' accumulation chains op-by-op on DVE so each
# op's pipeline drain overlaps the other tile's (independent) op.
INTERLEAVE = True

# PE offload: TensorE computes all 4 taps for columns [PE_SPLIT, T) via
# PSUM-accumulated diagonal fp32 matmuls (W = diag(Cj), shifts via the
# moving operand's AP column offset); DVE then merges g' = D - psum with
# one fused op per <=512-col PSUM bank. PE_SPLIT >= T disables.
PE_SPLIT = 2816
PE_BLOCK = 512
# PE_ADD_D: use negative diagonals and a 5th identity-diagonal matmul that
# adds D into PSUM, so PSUM holds g' directly and ScalarE (not DVE) copies
# it out. Frees DVE of the merge ops.
PE_ADD_D = True

_compiled = {}


def _build(steps: int):
    assert not (INTERLEAVE and SPLIT != T), "interleaved variant is DVE-only"
    pe_on = PE_SPLIT < T
    nc = bacc.Bacc("TRN2", target_bir_lowering=False, debug=False)

    x = nc.dram_tensor("x", [ROWS, T], F32, kind="ExternalInput").ap()
    coef = nc.dram_tensor("coef", [ROWS, 6], F32, kind="ExternalInput").ap()
    out = nc.dram_tensor("out", [ROWS, T], F32, kind="ExternalOutput").ap()
    if pe_on:
        wcols = 640 if PE_ADD_D else 512
        wdiag = nc.dram_tensor("wdiag", [ROWS, wcols], F32,
                               kind="ExternalInput").ap()
        wdiag_h = wdiag.rearrange("(h p) c -> h p c", p=128)

    x_h = x.rearrange("(h p) t -> h p t", p=128)
    out_h = out.rearrange("(h p) t -> h p t", p=128)
    coef_h = coef.rearrange("(h p) c -> h p c", p=128)

    mult = mybir.AluOpType.mult
    add = mybir.AluOpType.add

    with tile.TileContext(nc) as tc:
        with tc.tile_pool(name="state", bufs=1) as pool, \
             tc.tile_pool(name="psum", bufs=8, space="PSUM") as pspool:
            neg_half = pool.tile([128, 1], F32, tag="neg_half", name="neg_half")
            nc.vector.memset(neg_half[:], -0.5)
            gA, gB, sq, D, cf, tg = [], [], [], [], [], []
            for h in range(HALVES):
                gA.append(pool.tile([128, T], F32, tag=f"gA{h}", name=f"gA{h}"))
                gB.append(pool.tile([128, T], F32, tag=f"gB{h}", name=f"gB{h}"))
                sq.append(pool.tile([128, T + 3], F32, tag=f"sq{h}", name=f"sq{h}"))
                D.append(pool.tile([128, T], F32, tag=f"D{h}", name=f"D{h}"))
                cf.append(pool.tile([128, 6], F32, tag=f"cf{h}", name=f"cf{h}"))
                if SPLIT < T:
                    tg.append(pool.tile([128, T - SPLIT], F32, tag=f"tg{h}",
                                        name=f"tg{h}"))

            wd = []
            if pe_on:
                for h in range(HALVES):
                    wd.append(pool.tile([128, wcols], F32, tag=f"wd{h}",
                                        name=f"wd{h}"))
                    nc.sync.dma_start(out=wd[h][:], in_=wdiag_h[h])
            for h in range(HALVES):
                nc.sync.dma_start(out=cf[h][:], in_=coef_h[h])
                nc.sync.dma_start(out=gA[h][:], in_=x_h[h])
                # pad columns stay at a^2-of-zero = 0.25 forever
                nc.vector.memset(sq[h][:, 0:3], 0.25)
                # D = beta * x0 + dconst
                nc.vector.tensor_scalar(
                    D[h][:], gA[h][:], cf[h][:, 4:5], cf[h][:, 5:6], mult, add
                )

            def dve_ops(h, nxt, c0, c1):
                # g' = (sq[t]*negC3 + D) + sq[t-1]*negC2 + sq[t-2]*negC1
                #      + sq[t-3]*negC0   -- fused mult+add per tap
                nc.vector.scalar_tensor_tensor(
                    nxt[h][:, c0:c1], sq[h][:, 3 + c0:3 + c1], cf[h][:, 0:1],
                    D[h][:, c0:c1], mult, add,
                )
                for j, off in ((1, 2), (2, 1), (3, 0)):
                    nc.vector.scalar_tensor_tensor(
                        nxt[h][:, c0:c1], sq[h][:, off + c0:off + c1],
                        cf[h][:, j:j + 1], nxt[h][:, c0:c1], mult, add,
                    )

            def gp_ops(h, nxt, c0, c1):
                # GPSIMD has no scalar_tensor_tensor; per tap: tensor_scalar
                # mult (1-input, line rate) then tensor_tensor add.
                n = c1 - c0
                t = tg[h]
                nc.gpsimd.tensor_scalar(
                    t[:, :n], sq[h][:, 3 + c0:3 + c1], cf[h][:, 0:1], None, mult
                )
                nc.gpsimd.tensor_tensor(
                    nxt[h][:, c0:c1], t[:, :n], D[h][:, c0:c1], add
                )
                for j, off in ((1, 2), (2, 1), (3, 0)):
                    nc.gpsimd.tensor_scalar(
                        t[:, :n], sq[h][:, off + c0:off + c1], cf[h][:, j:j + 1],
                        None, mult,
                    )
                    nc.gpsimd.tensor_tensor(
                        nxt[h][:, c0:c1], t[:, :n], nxt[h][:, c0:c1], add
                    )

            dve_end = PE_SPLIT if pe_on else SPLIT
            pe_blocks = []
            c = PE_SPLIT
            while c < T:
                n = min(PE_BLOCK, T - c)
                pe_blocks.append((c, n))
                c += n

            for s in range(steps):
                cur, nxt = (gA, gB) if s % 2 == 0 else (gB, gA)
                # ACT squares for both tiles first, then the accumulation
                # chains — lets ACT(tile1) overlap DVE(tile0).
                for h in range(HALVES):
                    nc.scalar.activation(
                        sq[h][:, 3:3 + T], cur[h][:],
                        mybir.ActivationFunctionType.Square, bias=neg_half[:],
                    )
                # PE: psum[c0:c0+n] = sum_k diag(C_{3-k}) @ sq[:, off+c0 :]
                # (accumulated in-bank); DVE merges g' = D - psum later.
                step_psums = []
                if pe_on:
                    n_mm = 5 if PE_ADD_D else 4
                    for h in range(HALVES):
                        for (c0, n) in pe_blocks:
                            ps = pspool.tile([128, PE_BLOCK], F32, tag="ps",
                                             name=f"ps{s}_{h}_{c0}")
                            for k in range(4):
                                off = 3 - k
                                nc.tensor.matmul(
                                    ps[:, :n],
                                    wd[h][:, k * 128:(k + 1) * 128],
                                    sq[h][:, off + c0:off + c0 + n],
                                    start=(k == 0), stop=(k == n_mm - 1),
                                )
                            if PE_ADD_D:
                                nc.tensor.matmul(
                                    ps[:, :n], wd[h][:, 512:640],
                                    D[h][:, c0:c0 + n],
                                    start=False, stop=True,
                                )
                            step_psums.append((h, c0, n, ps))
                if INTERLEAVE:
                    # alternate the two tiles' chains op-by-op on DVE
                    for j, off in ((0, 3), (1, 2), (2, 1), (3, 0)):
                        for h in range(HALVES):
                            in1 = D[h] if j == 0 else nxt[h]
                            nc.vector.scalar_tensor_tensor(
                                nxt[h][:, 0:dve_end], sq[h][:, off:off + dve_end],
                                cf[h][:, j:j + 1], in1[:, 0:dve_end], mult, add,
                            )
                else:
                    for h in range(HALVES):
                        dve_ops(h, nxt, 0, dve_end)
                        if not pe_on and SPLIT < T:
                            gp_ops(h, nxt, SPLIT, T)
                # merges after the stt chains: psum is ready by now
                for (h, c0, n, ps) in step_psums:
                    if PE_ADD_D:
                        # psum already holds g'; ScalarE copies it out
                        nc.scalar.copy(nxt[h][:, c0:c0 + n], ps[:, :n])
                    else:
                        nc.vector.scalar_tensor_tensor(
                            nxt[h][:, c0:c0 + n], ps[:, :n], -1.0,
                            D[h][:, c0:c0 + n], mult, add,
                        )

            fin = gA if steps % 2 == 0 else gB
            for h in range(HALVES):
                nc.vector.tensor_scalar(
                    fin[h][:], fin[h][:], CLAMP, 1.0 - CLAMP,
                    mybir.AluOpType.max, mybir.AluOpType.min,
                )
                nc.sync.dma_start(out=out_h[h], in_=fin[h][:])

    nc.compile()
    return nc


def get_nc(steps: int):
    if steps not in _compiled:
        _compiled[steps] = _build(steps)
    return _compiled[steps]


def _host_prep(drive, r, eps, beta, K_causal):
    """Per-core input maps: x (256, T) and coef (256, 6)."""
    drive = np.asarray(drive, np.float32)
    r = np.asarray(r, np.float32)
    eps = np.asarray(eps, np.float32)
    beta = np.asarray(beta, np.float32)
    K = np.asarray(K_causal, np.float32)[:, 0, :]  # (C, 4)

    one_m_b = 1.0 - beta
    C0 = one_m_b * eps * r * K[:, 0]
    C1 = one_m_b * eps * r * K[:, 1]
    C2 = one_m_b * eps * r * K[:, 2]
    C3 = one_m_b * r * ((1.0 - eps) + eps * K[:, 3])
    dconst = 0.25 * (C0 + C1 + C2 + C3)

    pe_on = PE_SPLIT < T
    in_maps = []
    idx = np.arange(128)
    for i in range(N_CORES):
        sl = slice(i * CPC, (i + 1) * CPC)
        xs = np.ascontiguousarray(
            drive[:, :, sl].transpose(0, 2, 1).reshape(ROWS, T), np.float32
        )
        cs = np.stack(
            [np.tile(-C3[sl], B), np.tile(-C2[sl], B), np.tile(-C1[sl], B),
             np.tile(-C0[sl], B), np.tile(beta[sl], B), np.tile(dconst[sl], B)],
            axis=1,
        ).astype(np.float32)
        m = {"x": xs, "coef": np.ascontiguousarray(cs)}
        if pe_on:
            sign = -1.0 if PE_ADD_D else 1.0
            blocks = [sign * C3, sign * C2, sign * C1, sign * C0]
            if PE_ADD_D:
                blocks.append(np.ones(C, np.float32))
            wdg = np.zeros((ROWS, 128 * len(blocks)), np.float32)
            for k, arr in enumerate(blocks):
                rows = np.tile(np.asarray(arr, np.float32)[sl], B)  # (ROWS,)
                for h in range(HALVES):
                    wdg[h * 128 + idx, k * 128 + idx] = rows[h * 128 + idx]
            m["wdiag"] = wdg
        in_maps.append(m)
    return in_maps


def kernel(drive, r, eps, beta, K_causal, steps):
    steps = int(steps)
    nc = get_nc(steps)
    in_maps = _host_prep(drive, r, eps, beta, K_causal)
    res = run_bass_kernel_spmd(nc, in_maps, list(range(N_CORES)))
    parts = [
        res.results[i]["out"].reshape(B, CPC, T).transpose(0, 2, 1)
        for i in range(N_CORES)
    ]
    return np.ascontiguousarray(np.concatenate(parts, axis=2), np.float32)
